# revision 1
# baseline (speedup 1.0000x reference)
"""CRF negative log-likelihood on 8 Trainium2 NeuronCores.

Math (per batch column b, all in the normalized-probability domain):
  p_0 = exp(feats[0] + start)
  p_t = (E^T p_{t-1}) * 2^-S * exp(feats[t]),   E = exp(trans_m)
        (every KNORM steps additionally divided by the column sum s_e,
         whose log is tracked exactly via the stored reciprocal)
  all_path = log(sum_j exp(end_j) * p_{L-1}[j]) + (L-1)*S*ln2 + sum_e log s_e
  nll = all_path - gold_score

The tag-coupled scan runs on the PE (one [48x49]@[48x64] matmul per step,
ones column produces running column sums for the renormalization), the
PSUM->SBUF extraction is a single fused scalar_tensor_tensor on the vector
engine.  Gold emission sums run on the otherwise idle GPSIMD engine as a
fused (iota == tag) * feats accumulate.  The tag-table-only part of the gold
score (start/trans/end lookups) is index arithmetic on tiny inputs and is
folded in on the host during unsharding.
"""

import math
from contextlib import ExitStack

import numpy as np

import concourse.bass as bass
import concourse.bacc as bacc
import concourse.tile as tile
from concourse import mybir
from concourse.bass_utils import run_bass_kernel_spmd

B, L, T = 512, 1024, 48
NCORES = 8
BC = B // NCORES  # batch columns per core

S2 = 6  # constant per-step exponent shift (2**-S2 folded into the step op)
KNORM = 64  # true column renormalization period
TCH = 64  # time steps per streamed chunk

FP32 = mybir.dt.float32
BF16 = mybir.dt.bfloat16
I32 = mybir.dt.int32


def _build(l_steps: int = L, tch: int = TCH):
    n_events = (l_steps - 1) // KNORM
    nc = bacc.Bacc(
        "TRN2",
        target_bir_lowering=False,
        debug=False,
        num_devices=NCORES,
    )

    wfeats = nc.dram_tensor("wfeats", [l_steps, T, BC], FP32, kind="ExternalInput")
    nfeats = nc.dram_tensor("nfeats", [BC, l_steps * T], FP32, kind="ExternalInput")
    tags_d = nc.dram_tensor("tags_d", [BC, l_steps], I32, kind="ExternalInput")
    expm = nc.dram_tensor("expm", [T, T], FP32, kind="ExternalInput")
    startv = nc.dram_tensor("startv", [T, 1], FP32, kind="ExternalInput")
    expend = nc.dram_tensor("expend", [T, 1], FP32, kind="ExternalInput")
    out_a = nc.dram_tensor("out_a", [1, BC], FP32, kind="ExternalOutput")
    out_ge = nc.dram_tensor("out_ge", [BC, 1], FP32, kind="ExternalOutput")
    out_rc = nc.dram_tensor(
        "out_rc", [1, BC * max(n_events, 1)], FP32, kind="ExternalOutput"
    )

    n_chunks = (l_steps + tch - 1) // tch

    with tile.TileContext(nc) as tc, ExitStack() as ctx:
        singles = ctx.enter_context(tc.tile_pool(name="singles", bufs=1))
        wstage_p = ctx.enter_context(tc.tile_pool(name="wstage", bufs=2))
        wbf_p = ctx.enter_context(tc.tile_pool(name="wbf", bufs=2))
        nstage_p = ctx.enter_context(tc.tile_pool(name="nstage", bufs=2))
        nbf_p = ctx.enter_context(tc.tile_pool(name="nbf", bufs=2))
        p_pool = ctx.enter_context(tc.tile_pool(name="pstate", bufs=3))
        gd_pool = ctx.enter_context(tc.tile_pool(name="golddummy", bufs=2))
        psum_q = ctx.enter_context(tc.tile_pool(name="psq", bufs=2, space="PSUM"))
        psum_m = ctx.enter_context(tc.tile_pool(name="psm", bufs=1, space="PSUM"))

        # ---- constants ----
        # ones column lives at output partition 64 (engine partition starts
        # must be 32-aligned, so the column-sum row cannot sit at 48)
        SROW = 64
        expm_sb = singles.tile([T, T], FP32)
        nc.sync.dma_start(out=expm_sb, in_=expm.ap())
        e_aug = singles.tile([T, SROW + 1], BF16)
        nc.scalar.activation(e_aug[:, 0:T], expm_sb, mybir.ActivationFunctionType.Copy)
        nc.vector.memset(e_aug[:, T:SROW], 0.0)
        nc.vector.memset(e_aug[:, SROW : SROW + 1], 1.0)

        start_sb = singles.tile([T, 1], FP32)
        nc.sync.dma_start(out=start_sb, in_=startv.ap())
        expend_sb = singles.tile([T, 1], FP32)
        nc.sync.dma_start(out=expend_sb, in_=expend.ap())
        exp_end = singles.tile([T, 1], BF16)
        nc.vector.tensor_copy(exp_end, expend_sb)

        ones_row = singles.tile([1, T], FP32)
        nc.vector.memset(ones_row, 1.0)

        iota48 = singles.tile([BC, T], BF16)
        nc.gpsimd.iota(
            iota48,
            pattern=[[1, T]],
            base=0,
            channel_multiplier=0,
            allow_small_or_imprecise_dtypes=True,
        )

        tags_sb = singles.tile([BC, l_steps], I32)
        nc.sync.dma_start(out=tags_sb, in_=tags_d.ap())
        tags_bf = singles.tile([BC, l_steps], BF16)
        nc.vector.tensor_copy(tags_bf, tags_sb)

        gebuf = singles.tile([BC, l_steps], FP32)
        recips = singles.tile([1, BC, max(n_events, 1)], FP32)

        out_a_sb = singles.tile([1, BC], FP32)
        out_ge_sb = singles.tile([BC, 1], FP32)

        p_cur = None
        ev_idx = 0
        for ich in range(n_chunks):
            t0 = ich * tch
            tn = min(tch, l_steps - t0)
            # streamed loads: time-major (exp'd weights) + natural (gold)
            wstage = wstage_p.tile([T, tch, BC], FP32, tag="wstage")
            nc.sync.dma_start(
                out=wstage[:, 0:tn, :],
                in_=wfeats.ap()[t0 : t0 + tn].rearrange("t j b -> j t b"),
            )
            wbf = wbf_p.tile([T, tch, BC], BF16, tag="wbf")
            nc.scalar.activation(
                wbf[:, 0:tn, :],
                wstage[:, 0:tn, :],
                mybir.ActivationFunctionType.Exp,
            )
            nstage = nstage_p.tile([BC, tch, T], FP32, tag="nstage")
            nc.sync.dma_start(
                out=nstage[:, 0:tn, :],
                in_=nfeats.ap()[:, t0 * T : (t0 + tn) * T].rearrange(
                    "b (t j) -> b t j", j=T
                ),
            )
            nbf = nbf_p.tile([BC, tch, T], BF16, tag="nbf")
            nc.scalar.activation(
                nbf[:, 0:tn, :],
                nstage[:, 0:tn, :],
                mybir.ActivationFunctionType.Copy,
            )

            for trel in range(tn):
                t = t0 + trel
                if t == 0:
                    p_cur = p_pool.tile([T, BC], BF16, tag="p")
                    nc.scalar.activation(
                        p_cur,
                        wstage[:, 0, :],
                        mybir.ActivationFunctionType.Exp,
                        bias=start_sb,
                    )
                else:
                    q = psum_q.tile([SROW + 1, BC], FP32, tag="q")
                    nc.tensor.matmul(q, e_aug, p_cur, start=True, stop=True)
                    p_new = p_pool.tile([T, BC], BF16, tag="p")
                    nc.vector.scalar_tensor_tensor(
                        out=p_new,
                        in0=q[0:T, :],
                        scalar=2.0 ** (-S2),
                        in1=wbf[:, trel, :],
                        op0=mybir.AluOpType.mult,
                        op1=mybir.AluOpType.mult,
                    )
                    p_cur = p_new
                    if t % KNORM == 0 and t >= KNORM and ev_idx < n_events:
                        rc = recips[:, :, ev_idx]
                        nc.vector.reciprocal(rc, q[SROW : SROW + 1, :])
                        bq = psum_m.tile([T, BC], FP32, tag="bc")
                        nc.tensor.matmul(bq, ones_row, rc, start=True, stop=True)
                        p_nrm = p_pool.tile([T, BC], BF16, tag="p")
                        nc.vector.scalar_tensor_tensor(
                            out=p_nrm,
                            in0=bq,
                            scalar=1.0,
                            in1=p_cur,
                            op0=mybir.AluOpType.mult,
                            op1=mybir.AluOpType.mult,
                        )
                        p_cur = p_nrm
                        ev_idx += 1

                # gold emission for step t (fills DVE gaps in the serial chain)
                gd = gd_pool.tile([BC, T], BF16, tag="gd")
                nc.vector.scalar_tensor_tensor(
                    out=gd,
                    in0=iota48,
                    scalar=tags_bf[:, t : t + 1],
                    in1=nbf[:, trel, :],
                    op0=mybir.AluOpType.is_equal,
                    op1=mybir.AluOpType.mult,
                    accum_out=gebuf[:, t : t + 1],
                )

        # ---- final combine (logs happen on host, in f64) ----
        fin = psum_m.tile([1, BC], FP32, tag="fin")
        nc.tensor.matmul(fin, exp_end, p_cur, start=True, stop=True)
        nc.vector.tensor_copy(out_a_sb, fin)
        nc.sync.dma_start(out=out_a.ap(), in_=out_a_sb)
        if n_events > 0:
            nc.sync.dma_start(
                out=out_rc.ap(),
                in_=recips[:, :, 0:n_events].rearrange("p b e -> p (b e)"),
            )
        else:
            nc.sync.dma_start(out=out_rc.ap(), in_=recips[:, :, 0])

        nc.vector.tensor_reduce(
            out_ge_sb, gebuf, axis=mybir.AxisListType.X, op=mybir.AluOpType.add
        )
        nc.sync.dma_start(out=out_ge.ap(), in_=out_ge_sb)

    nc.compile()
    return nc


def _host_prep(feats, tags, l_steps):
    """Per-core input dicts: batch-shard + time-major transpose."""
    in_maps = []
    for c in range(NCORES):
        sl = slice(c * BC, (c + 1) * BC)
        f = feats[sl]  # [BC, L, T]
        in_maps.append(
            {
                "wfeats": np.ascontiguousarray(f.transpose(1, 2, 0)),
                "nfeats": np.ascontiguousarray(f.reshape(BC, l_steps * T)),
                "tags_d": np.ascontiguousarray(tags[sl]),
            }
        )
    return in_maps


def kernel(feats, tags, mask, trans_m, start_scores, end_scores):
    feats = np.asarray(feats, dtype=np.float32)
    tags = np.asarray(tags, dtype=np.int32)
    trans_m = np.asarray(trans_m, dtype=np.float32)
    start_scores = np.asarray(start_scores, dtype=np.float32)
    end_scores = np.asarray(end_scores, dtype=np.float32)

    nc = _build(L, TCH)
    in_maps = _host_prep(feats, tags, L)
    for m in in_maps:
        m["expm"] = np.exp(trans_m.astype(np.float64)).astype(np.float32)
        m["startv"] = start_scores.reshape(T, 1)
        m["expend"] = np.exp(end_scores.astype(np.float64)).astype(np.float32).reshape(T, 1)

    res = run_bass_kernel_spmd(nc, in_maps, list(range(NCORES)))
    return _host_finish(res.results, tags, trans_m, start_scores, end_scores, L)


def _host_finish(results, tags, trans_m, start_scores, end_scores, l_steps):
    """Unshard + exact log bookkeeping + tag-table gold terms (f64)."""
    n_events = (l_steps - 1) // KNORM
    const = (l_steps - 1) * S2 * math.log(2.0)
    gold_tab = (
        start_scores[tags[:, 0]].astype(np.float64)
        + trans_m.astype(np.float64)[tags[:, :-1], tags[:, 1:]].sum(axis=1)
        + end_scores[tags[:, -1]].astype(np.float64)
    )

    out = np.empty(B, dtype=np.float64)
    for c in range(NCORES):
        sl = slice(c * BC, (c + 1) * BC)
        fin = results[c]["out_a"].reshape(BC).astype(np.float64)
        all_path = np.log(fin) + const
        if n_events > 0:
            rc = results[c]["out_rc"].reshape(BC, n_events).astype(np.float64)
            all_path -= np.log(rc).sum(axis=1)
        ge = results[c]["out_ge"].reshape(BC).astype(np.float64)
        out[sl] = all_path - ge - gold_tab[sl]
    return out.astype(np.float32)



# revision 5
# speedup vs baseline: 4.0297x; 4.0297x over previous
"""CRF negative log-likelihood on 8 Trainium2 NeuronCores.

Strategy (v2): the forward DP over L=1024 steps is a serial chain of
(48x48 matmul -> elementwise emission multiply) whose per-step latency is
dominated by fixed HW handoff costs (PE drain, DVE<->PSUM access, sem
propagation).  Batch-splitting cannot shorten it, so we shard TIME:
the CRF forward recursion forgets its initial state at ~1e-10 per 16
steps (positive-matrix mixing), so the 1023 steps are cut into 8
segments, each recomputed from a 16-step burn-in that starts at a
uniform-ish vector.  8 cores = 2 batch shards x 4 time quarters; each
core runs its 2 segments as interleaved streams (PE tiles (0,0)/(64,64)
via tile_position), giving ~142 serial hops instead of 1023.

Per hop: one 48x48xCOLS matmul per stream (weights resident in the PE
array: 4 ldweights at startup, every matmul has ldweights=False) and one
DVE scalar_tensor_tensor that drains PSUM, multiplies by exp(feats_t)
(host-precomputed bf16) and by 2^-S2 to keep magnitudes bounded.  Every
~64 hops a ones-weights matmul (tiles (0,64)/(64,0)) produces the column
sums replicated across partitions; their reciprocal renormalizes the
state by premultiplying the emission slice two hops ahead, and the raw
colsums go to the host, which reassembles log Z exactly (the 2^-S2 and
renorm corrections are pure bookkeeping).  start/end scores are folded
into the first/last emission slice on the host; the gold-path score is
computed entirely on the host in float64.
"""

import math
from contextlib import ExitStack

import numpy as np

import concourse.bacc as bacc
import concourse.tile as tile
from concourse import mybir
from concourse.bass_utils import run_bass_kernel_spmd

B, L, T = 512, 1024, 48
NCORES = 8

SB = 2                # batch shards
COLS = B // SB        # 256 columns per core
NSEG = 8              # global time segments (= 4 time-parts x 2 streams)
ETA = 16              # burn-in steps for segments 1..7
H = 142               # matmul hops per stream (slices 1..H; slice 0 = init)
NSLICE = H + 1
TCH = 32              # w slices per DMA chunk
NCH = (NSLICE + TCH - 1) // TCH   # 5 chunks (slices padded to NCH*TCH)
S2 = 7                # per-hop 2^-S2 scaling (log2 colsum mean ~7.03)
EVENTS = (16, 80, 141, 142)       # colsum measure hops
APPLY = (16, 80)                  # events whose 1/colsum rescales slice j+2
NEV = len(EVENTS)
SEG0_LEN = 142        # segment 0: steps 1..142, exact from f_0
SEG_LEN = 126         # segments 1..6: 126 real steps; segment 7: 125

FP32 = mybir.dt.float32
BF16 = mybir.dt.bfloat16

# stream -> (state/weight partition base, colsum partition base)
STREAMS = ((0, 64), (64, 0))


def _seg_start(s):
    """First real step of segment s (1-based step index)."""
    return 1 if s == 0 else SEG0_LEN + 1 + SEG_LEN * (s - 1)


def _build(use_ldw_elision=True):
    nc = bacc.Bacc(
        "TRN2",
        target_bir_lowering=False,
        debug=False,
        num_devices=NCORES,
    )

    wbuf = nc.dram_tensor("wbuf", [NCH * 96, TCH * COLS], BF16, kind="ExternalInput")
    wts = nc.dram_tensor("wts", [128, 64], BF16, kind="ExternalInput")
    onesw = nc.dram_tensor("onesw", [128, 64], BF16, kind="ExternalInput")
    out_cs = nc.dram_tensor("out_cs", [2, NEV * COLS], FP32, kind="ExternalOutput")

    with tile.TileContext(nc) as tc, ExitStack() as ctx:
        singles = ctx.enter_context(tc.tile_pool(name="singles", bufs=1))
        wpool = ctx.enter_context(tc.tile_pool(name="wpool", bufs=2))
        ppools = [
            ctx.enter_context(tc.tile_pool(name=f"p{s}", bufs=3)) for s in range(2)
        ]
        pspools = [
            ctx.enter_context(tc.tile_pool(name=f"ps{s}", bufs=2, space="PSUM"))
            for s in range(2)
        ]
        csps = ctx.enter_context(tc.tile_pool(name="csps", bufs=1, space="PSUM"))
        rcpool = ctx.enter_context(tc.tile_pool(name="rc", bufs=2))

        e_sb = singles.tile([128, 64], BF16)
        nc.sync.dma_start(out=e_sb, in_=wts.ap())
        o_sb = singles.tile([128, 64], BF16)
        nc.sync.dma_start(out=o_sb, in_=onesw.ap())
        # stream s's colsums live on partition 32*s (engine starts 32-aligned)
        cs_sb = singles.tile([64, NEV * COLS], FP32)

        # Load the four persistent weight tiles (64x64 array mode):
        # (0,0)/(64,64): E for streams 0/1; (0,64)/(64,0): ones for colsums.
        nc.tensor.ldweights(e_sb[0:64, :], tile_position=(0, 0))
        nc.tensor.ldweights(e_sb[64:128, :], tile_position=(64, 64))
        nc.tensor.ldweights(o_sb[0:64, :], tile_position=(0, 64))
        nc.tensor.ldweights(o_sb[64:128, :], tile_position=(64, 0))

        wt = [None] * NCH

        def wsl(ch_tile, r0, j):
            pos = j % TCH
            return ch_tile[r0 : r0 + 48, pos * COLS : (pos + 1) * COLS]

        p_cur = [None, None]
        for j in range(0, H + 1):
            ch, pos = divmod(j, TCH)
            if pos == 0:
                w = wpool.tile([128, TCH * COLS], BF16, tag="w")
                nc.sync.dma_start(
                    out=w[0:48, :], in_=wbuf.ap()[ch * 96 : ch * 96 + 48, :]
                )
                nc.sync.dma_start(
                    out=w[64:112, :], in_=wbuf.ap()[ch * 96 + 48 : ch * 96 + 96, :]
                )
                wt[ch] = w

            for s, (r0, cb) in enumerate(STREAMS):
                if j == 0:
                    p = ppools[s].tile([128, COLS], BF16, tag=f"p{s}")
                    nc.vector.tensor_copy(p[r0 : r0 + 48, :], wsl(wt[0], r0, 0))
                    p_cur[s] = p
                    continue

                q = pspools[s].tile([128, COLS], FP32, tag=f"q{s}")
                mm = nc.tensor.matmul(
                    q[r0 : r0 + 48, :],
                    e_sb[r0 : r0 + 48, 0:48],
                    p_cur[s][r0 : r0 + 48, :],
                    start=True,
                    stop=True,
                )
                if use_ldw_elision:
                    mm.ins.ldweights = False

                pn = ppools[s].tile([128, COLS], BF16, tag=f"p{s}")
                nc.vector.scalar_tensor_tensor(
                    out=pn[r0 : r0 + 48, :],
                    in0=q[r0 : r0 + 48, :],
                    scalar=2.0 ** (-S2),
                    in1=wsl(wt[ch], r0, j),
                    op0=mybir.AluOpType.mult,
                    op1=mybir.AluOpType.mult,
                )
                p_cur[s] = pn

                if j in EVENTS:
                    k = EVENTS.index(j)
                    cq = csps.tile([128, COLS], FP32, tag=f"cs{s}")
                    cmm = nc.tensor.matmul(
                        cq[cb : cb + 48, :],
                        o_sb[r0 : r0 + 48, 0:48],
                        pn[r0 : r0 + 48, :],
                        start=True,
                        stop=True,
                    )
                    if use_ldw_elision:
                        cmm.ins.ldweights = False
                    nc.vector.tensor_copy(
                        cs_sb[32 * s : 32 * s + 1, k * COLS : (k + 1) * COLS],
                        cq[cb : cb + 1, :],
                    )
                    if j in APPLY:
                        rc = rcpool.tile([128, COLS], FP32, tag=f"rc{s}")
                        nc.vector.reciprocal(rc[r0 : r0 + 48, :], cq[cb : cb + 48, :])
                        tgt = wsl(wt[(j + 2) // TCH], r0, j + 2)
                        nc.vector.scalar_tensor_tensor(
                            out=tgt,
                            in0=tgt,
                            scalar=1.0,
                            in1=rc[r0 : r0 + 48, :],
                            op0=mybir.AluOpType.mult,
                            op1=mybir.AluOpType.mult,
                        )

        nc.sync.dma_start(out=out_cs.ap()[0:1, :], in_=cs_sb[0:1, :])
        nc.sync.dma_start(out=out_cs.ap()[1:2, :], in_=cs_sb[32:33, :])

    # Excess matmul waits must become sync-queue event semaphores, not get
    # pinned onto the four startup ldweights (which would deadlock: they
    # execute first on the in-order PE queue).
    nc.move_matmul_waits_to_ldweights = lambda: None
    nc.compile()
    return nc


def _host_prep(feats, trans, start, end):
    """Per-core input dicts: emission slices per (core, stream, hop)."""
    import ml_dtypes

    bf16 = ml_dtypes.bfloat16
    E = np.exp(trans.astype(np.float64)).astype(np.float32)
    wts = np.zeros((128, 64), np.float32)
    wts[0:48, 0:48] = E
    wts[64:112, 0:48] = E
    onesw = np.zeros((128, 64), np.float32)
    onesw[0:48, 0:48] = 1.0
    onesw[64:112, 0:48] = 1.0
    wts = wts.astype(bf16)
    onesw = onesw.astype(bf16)

    in_maps = []
    for c in range(NCORES):
        sh, tau = c // 4, c % 4
        colsl = slice(sh * COLS, (sh + 1) * COLS)
        f = feats[colsl]  # [COLS, L, T] float32
        # arr[slice j, stream block, tag, col]
        arr = np.ones((NCH * TCH, 2, T, COLS), np.float32)
        for srow in range(2):
            seg = 2 * tau + srow
            a = _seg_start(seg)
            t0 = 0 if seg == 0 else a - ETA - 1
            for j in range(NSLICE):
                t = t0 + j
                if t > L - 1:
                    continue  # padded (all ones)
                sl = f[:, t, :].astype(np.float64)
                if seg == 0 and j == 0:
                    sl = sl + start.astype(np.float64)
                if seg == 7 and t == L - 1:
                    sl = sl + end.astype(np.float64)
                arr[j, srow] = np.exp(sl).T.astype(np.float32)
        # -> [NCH, TCH, 2, 48, COLS] -> [NCH, 2, 48, TCH, COLS] -> flat
        a4 = arr.reshape(NCH, TCH, 2, T, COLS).transpose(0, 2, 3, 1, 4)
        wb = np.ascontiguousarray(a4).astype(bf16).reshape(NCH * 96, TCH * COLS)
        in_maps.append({"wbuf": wb, "wts": wts, "onesw": onesw})
    return in_maps


def _host_finish(results, feats, tags, trans, start, end):
    """Assemble log Z from colsums + exact gold score; returns NLL [B]."""
    c2 = S2 * math.log(2.0)
    logz = np.zeros(B, dtype=np.float64)
    for c in range(NCORES):
        sh, tau = c // 4, c % 4
        colsl = slice(sh * COLS, (sh + 1) * COLS)
        cs = results[c]["out_cs"].reshape(2, NEV, COLS).astype(np.float64)
        for srow in range(2):
            seg = 2 * tau + srow
            ln0 = np.log(cs[srow, 0])
            ln1 = np.log(cs[srow, 1])
            if seg == 7:
                lend = 141 * c2 + ln0 + ln1 + np.log(cs[srow, 2])
            else:
                lend = 142 * c2 + ln0 + ln1 + np.log(cs[srow, 3])
            bound = 0.0 if seg == 0 else 16 * c2 + ln0
            logz[colsl] += lend - bound

    f = feats.astype(np.float64)
    emit = np.take_along_axis(f, tags[:, :, None].astype(np.int64), axis=2)[:, :, 0]
    gold = (
        emit.sum(axis=1)
        + trans.astype(np.float64)[tags[:, :-1], tags[:, 1:]].sum(axis=1)
        + start.astype(np.float64)[tags[:, 0]]
        + end.astype(np.float64)[tags[:, -1]]
    )
    return (logz - gold).astype(np.float32)


def kernel(feats, tags, mask, trans_m, start_scores, end_scores):
    feats = np.asarray(feats, dtype=np.float32)
    tags = np.asarray(tags, dtype=np.int32)
    trans_m = np.asarray(trans_m, dtype=np.float32)
    start_scores = np.asarray(start_scores, dtype=np.float32)
    end_scores = np.asarray(end_scores, dtype=np.float32)

    nc = _build()
    in_maps = _host_prep(feats, trans_m, start_scores, end_scores)
    res = run_bass_kernel_spmd(nc, in_maps, list(range(NCORES)))
    return _host_finish(res.results, feats, tags, trans_m, start_scores, end_scores)


# revision 8
# speedup vs baseline: 6.0164x; 1.4930x over previous
"""CRF negative log-likelihood on 8 Trainium2 NeuronCores.

Strategy (v3): the forward DP over L=1024 steps is a serial chain of
(48x48 matmul -> elementwise emission multiply) whose per-step wall time
(~900ns) is pinned by fixed HW handoff latency (PE drain, DVE<->PSUM
access, semaphore propagation) plus one DVE PSUM-drain per step.  Batch
splitting cannot shorten the chain, so we shard TIME: the CRF forward
recursion forgets its initial state at ~1e-10 per 16 steps (positive
matrix mixing), so the 1023 steps are cut into 16 segments, each
recomputed from a 16-step burn-in that starts at exp(feats) of the
preceding step.  8 cores = 2 batch shards x 4 time quarters; each core
runs its 4 segments as interleaved streams -> ~79 serial hops per core
instead of 1023.

Streams pair up: the pair's two matmuls use persistent PE weight tiles
(0,0)/(64,64) (tile_position array packing; the per-matmul LDWEIGHTS
carries no waits and overlaps the matmul) and write one shared PSUM
tile at partitions 0-48/64-112, drained by a SINGLE DVE
scalar_tensor_tensor that also multiplies exp(feats_t) (host-built
bf16) and 2^-S2 (magnitude control; with ~79-hop segments no other
renormalization is needed -- log2 mass stays in [1,16]).  The E weights
carry a fused ones-column, so row 48/112 of every matmul output is the
column sum; boundary/end events just copy it out, and the host
reassembles log Z exactly.  start/end scores fold into the first/last
emission slice; the gold-path score is pure host-side float64.
"""

import math
from contextlib import ExitStack

import numpy as np

import concourse.bacc as bacc
import concourse.tile as tile
from concourse import mybir
from concourse.bass_utils import run_bass_kernel_spmd

B, L, T = 512, 1024, 48
NCORES = 8

SB = 2                 # batch shards
COLS = B // SB         # 256 columns per core
NSEG = 16              # global time segments (4 time-parts x 4 streams)
NPAIR = 2              # stream pairs per core
ETA = 16               # burn-in steps (segments 1..15)
H = 79                 # matmul hops per stream (slice 0 = init)
NSLICE = H + 1
TCH = 32               # w slices per DMA chunk
NCH = (NSLICE + TCH - 1) // TCH   # 3
S2 = 7                 # per-hop 2^-S2 scaling (log2 colsum mean ~7.03)
EVENTS = (16, 78, 79)  # colsum measure hops (boundary, end(seg15), end)
NEV = len(EVENTS)
SEG0_LEN = 79          # segment 0: steps 1..79 exact from f_0
SEG_LEN = 63           # segments 1..14: 63 real steps; segment 15: 62

FP32 = mybir.dt.float32
BF16 = mybir.dt.bfloat16


def _seg_start(s):
    """First real step of segment s (1-based step index)."""
    return 1 if s == 0 else SEG0_LEN + 1 + SEG_LEN * (s - 1)


def _build():
    nc = bacc.Bacc(
        "TRN2",
        target_bir_lowering=False,
        debug=False,
        num_devices=NCORES,
    )

    wbuf = nc.dram_tensor(
        "wbuf", [NPAIR * NCH * 128, TCH * COLS], BF16, kind="ExternalInput"
    )
    wts = nc.dram_tensor("wts", [128, 64], BF16, kind="ExternalInput")
    # colsums: per stream 0..3, NEV events each
    out_cs = nc.dram_tensor("out_cs", [1, 4 * NEV * COLS], FP32, kind="ExternalOutput")

    with tile.TileContext(nc) as tc, ExitStack() as ctx:
        singles = ctx.enter_context(tc.tile_pool(name="singles", bufs=1))
        wpools = [
            ctx.enter_context(tc.tile_pool(name=f"w{k}", bufs=2)) for k in range(NPAIR)
        ]
        ppools = [
            ctx.enter_context(tc.tile_pool(name=f"p{k}", bufs=3)) for k in range(NPAIR)
        ]
        pspools = [
            ctx.enter_context(tc.tile_pool(name=f"ps{k}", bufs=2, space="PSUM"))
            for k in range(NPAIR)
        ]

        e_sb = singles.tile([128, 64], BF16)
        nc.sync.dma_start(out=e_sb, in_=wts.ap())
        # events staging: sum rows land on partition 48 (copy window 32..48)
        stage = singles.tile([64, 4 * NEV * COLS], FP32)

        # Two persistent weight tiles (64x64 array mode): E+ones-col, both
        # stream slots.  Every matmul reuses them via tile_position.
        nc.tensor.ldweights(e_sb[0:64, :], tile_position=(0, 0))
        nc.tensor.ldweights(e_sb[64:128, :], tile_position=(64, 64))

        wt = [[None] * NCH for _ in range(NPAIR)]

        p_cur = [None] * NPAIR
        for j in range(0, H + 1):
            ch, pos = divmod(j, TCH)
            for k in range(NPAIR):
                if pos == 0 and wt[k][ch] is None:
                    w = wpools[k].tile([128, TCH * COLS], BF16, tag=f"w{k}")
                    r0 = (k * NCH + ch) * 128
                    nc.sync.dma_start(out=w, in_=wbuf.ap()[r0 : r0 + 128, :])
                    wt[k][ch] = w

                wsl = wt[k][ch][0:112, pos * COLS : (pos + 1) * COLS]
                if j == 0:
                    p = ppools[k].tile([128, COLS], BF16, tag=f"p{k}")
                    nc.vector.tensor_copy(p[0:112, :], wsl)
                    p_cur[k] = p
                    continue

                q = pspools[k].tile([128, COLS], FP32, tag=f"q{k}")
                nc.tensor.matmul(
                    q[0:64, :],
                    e_sb[0:48, 0:64],
                    p_cur[k][0:48, :],
                    start=True,
                    stop=True,
                ).ins.ldweights = False
                nc.tensor.matmul(
                    q[64:128, :],
                    e_sb[64:112, 0:64],
                    p_cur[k][64:112, :],
                    start=True,
                    stop=True,
                ).ins.ldweights = False

                pn = ppools[k].tile([128, COLS], BF16, tag=f"p{k}")
                nc.vector.scalar_tensor_tensor(
                    out=pn[0:112, :],
                    in0=q[0:112, :],
                    scalar=2.0 ** (-S2),
                    in1=wsl,
                    op0=mybir.AluOpType.mult,
                    op1=mybir.AluOpType.mult,
                )
                p_cur[k] = pn

                if j in EVENTS:
                    ev = EVENTS.index(j)
                    for half, base in ((0, 32), (1, 96)):
                        # stream 2k+half's colsum is q row 48/112; copy the
                        # 32-aligned window [base, base+17) -> rows 32..48.
                        sidx = 2 * k + half
                        off = (sidx * NEV + ev) * COLS
                        nc.vector.tensor_copy(
                            stage[32:49, off : off + COLS],
                            q[base : base + 17, :],
                        )

        nc.sync.dma_start(out=out_cs.ap(), in_=stage[48:49, :])

    # Excess matmul waits must become sync-queue event semaphores, not get
    # pinned onto the startup ldweights (in-order PE queue would deadlock).
    nc.move_matmul_waits_to_ldweights = lambda: None
    nc.compile()
    return nc


def _host_prep(feats, trans, start, end):
    """Per-core input dicts: emission slices per (core, stream, hop)."""
    import ml_dtypes

    bf16 = ml_dtypes.bfloat16
    E = np.exp(trans.astype(np.float64)).astype(np.float32)
    wts = np.zeros((128, 64), np.float32)
    wts[0:48, 0:48] = E
    wts[0:48, 48] = 1.0
    wts[64:112, 0:48] = E
    wts[64:112, 48] = 1.0
    wts = wts.astype(bf16)

    in_maps = []
    for c in range(NCORES):
        sh, tau = c // 4, c % 4
        colsl = slice(sh * COLS, (sh + 1) * COLS)
        f = feats[colsl]  # [COLS, L, T] float32
        # arr[slice j, stream, tag, col]
        arr = np.ones((NCH * TCH, 4, T, COLS), np.float32)
        for sidx in range(4):
            seg = 4 * tau + sidx
            a = _seg_start(seg)
            t0 = 0 if seg == 0 else a - ETA - 1
            for j in range(NSLICE):
                t = t0 + j
                if t > L - 1:
                    continue  # padded (all ones)
                sl = f[:, t, :].astype(np.float64)
                if seg == 0 and j == 0:
                    sl = sl + start.astype(np.float64)
                if seg == NSEG - 1 and t == L - 1:
                    sl = sl + end.astype(np.float64)
                arr[j, sidx] = np.exp(sl).T.astype(np.float32)
        # device rows per (pair, chunk): stream 2k at 0-47, 2k+1 at 64-111,
        # zero padding at 48-63/112-127 (keeps sim-visible SBUF initialized
        # and NaN-free garbage lanes) -> [NPAIR, NCH, 128, TCH, COLS]
        a4 = arr.reshape(NCH, TCH, 2, 2, T, COLS).transpose(2, 0, 3, 4, 1, 5)
        full = np.zeros((NPAIR, NCH, 2, 64, TCH, COLS), np.float32)
        full[:, :, :, 0:48] = a4
        wb = (
            np.ascontiguousarray(full)
            .astype(bf16)
            .reshape(NPAIR * NCH * 128, TCH * COLS)
        )
        in_maps.append({"wbuf": wb, "wts": wts})
    return in_maps


def _host_finish(results, feats, tags, trans, start, end):
    """Assemble log Z from colsums + exact gold score; returns NLL [B]."""
    c2 = S2 * math.log(2.0)
    logz = np.zeros(B, dtype=np.float64)
    for c in range(NCORES):
        sh, tau = c // 4, c % 4
        colsl = slice(sh * COLS, (sh + 1) * COLS)
        cs = results[c]["out_cs"].reshape(4, NEV, COLS).astype(np.float64)
        for sidx in range(4):
            seg = 4 * tau + sidx
            if seg == NSEG - 1:
                lend = 78 * c2 + np.log(cs[sidx, 1])
            else:
                lend = 79 * c2 + np.log(cs[sidx, 2])
            bound = 0.0 if seg == 0 else ETA * c2 + np.log(cs[sidx, 0])
            logz[colsl] += lend - bound

    f = feats.astype(np.float64)
    emit = np.take_along_axis(f, tags[:, :, None].astype(np.int64), axis=2)[:, :, 0]
    gold = (
        emit.sum(axis=1)
        + trans.astype(np.float64)[tags[:, :-1], tags[:, 1:]].sum(axis=1)
        + start.astype(np.float64)[tags[:, 0]]
        + end.astype(np.float64)[tags[:, -1]]
    )
    return (logz - gold).astype(np.float32)


def kernel(feats, tags, mask, trans_m, start_scores, end_scores):
    feats = np.asarray(feats, dtype=np.float32)
    tags = np.asarray(tags, dtype=np.int32)
    trans_m = np.asarray(trans_m, dtype=np.float32)
    start_scores = np.asarray(start_scores, dtype=np.float32)
    end_scores = np.asarray(end_scores, dtype=np.float32)

    nc = _build()
    in_maps = _host_prep(feats, trans_m, start_scores, end_scores)
    res = run_bass_kernel_spmd(nc, in_maps, list(range(NCORES)))
    return _host_finish(res.results, feats, tags, trans_m, start_scores, end_scores)


# revision 9
# speedup vs baseline: 6.5666x; 1.0915x over previous
"""CRF negative log-likelihood on 8 Trainium2 NeuronCores.

Strategy (v3): the forward DP over L=1024 steps is a serial chain of
(48x48 matmul -> elementwise emission multiply) whose per-step wall time
(~900ns) is pinned by fixed HW handoff latency (PE drain, DVE<->PSUM
access, semaphore propagation) plus one DVE PSUM-drain per step.  Batch
splitting cannot shorten the chain, so we shard TIME: the CRF forward
recursion forgets its initial state at ~1e-10 per 16 steps (positive
matrix mixing), so the 1023 steps are cut into 16 segments, each
recomputed from a 16-step burn-in that starts at exp(feats) of the
preceding step.  8 cores = 2 batch shards x 4 time quarters; each core
runs its 4 segments as interleaved streams -> ~79 serial hops per core
instead of 1023.

Streams pair up: the pair's two matmuls use persistent PE weight tiles
(0,0)/(64,64) (tile_position array packing; the per-matmul LDWEIGHTS
carries no waits and overlaps the matmul) and write one shared PSUM
tile at partitions 0-48/64-112, drained by a SINGLE DVE
scalar_tensor_tensor that also multiplies exp(feats_t) (host-built
bf16) and 2^-S2 (magnitude control; with ~79-hop segments no other
renormalization is needed -- log2 mass stays in [1,16]).  The E weights
carry a fused ones-column, so row 48/112 of every matmul output is the
column sum; boundary/end events just copy it out, and the host
reassembles log Z exactly.  start/end scores fold into the first/last
emission slice; the gold-path score is pure host-side float64.
"""

import math
from contextlib import ExitStack

import numpy as np

import concourse.bacc as bacc
import concourse.tile as tile
from concourse import mybir
from concourse.bass_utils import run_bass_kernel_spmd

B, L, T = 512, 1024, 48
NCORES = 8

SB = 2                 # batch shards
COLS = B // SB         # 256 columns per core
NSEG = 16              # global time segments (4 time-parts x 4 streams)
NPAIR = 2              # stream pairs per core
ETA = 12               # burn-in steps (segments 1..15)
H = 76                 # matmul hops per stream (slice 0 = init)
NSLICE = H + 1
TCH = 32               # w slices per DMA chunk
NCH = (NSLICE + TCH - 1) // TCH   # 3
S2 = 7                 # per-hop 2^-S2 scaling (log2 colsum mean ~7.03)
EVENTS = (12, 63, 76)  # colsum measure hops (boundary, end(seg15), end)
NEV = len(EVENTS)
SEG0_LEN = 76          # segment 0: steps 1..76 exact from f_0
SEG_LEN = 64           # segments 1..14: 64 real steps; segment 15: 51

FP32 = mybir.dt.float32
BF16 = mybir.dt.bfloat16


def _seg_start(s):
    """First real step of segment s (1-based step index)."""
    return 1 if s == 0 else SEG0_LEN + 1 + SEG_LEN * (s - 1)


def _build():
    nc = bacc.Bacc(
        "TRN2",
        target_bir_lowering=False,
        debug=False,
        num_devices=NCORES,
    )

    wbuf = nc.dram_tensor(
        "wbuf", [NPAIR * NCH * 128, TCH * COLS], BF16, kind="ExternalInput"
    )
    wts = nc.dram_tensor("wts", [128, 64], BF16, kind="ExternalInput")
    # colsums: per stream 0..3, NEV events each
    out_cs = nc.dram_tensor("out_cs", [1, 4 * NEV * COLS], FP32, kind="ExternalOutput")

    with tile.TileContext(nc) as tc, ExitStack() as ctx:
        singles = ctx.enter_context(tc.tile_pool(name="singles", bufs=1))
        wpools = [
            ctx.enter_context(tc.tile_pool(name=f"w{k}", bufs=2)) for k in range(NPAIR)
        ]
        ppools = [
            ctx.enter_context(tc.tile_pool(name=f"p{k}", bufs=3)) for k in range(NPAIR)
        ]
        pspools = [
            ctx.enter_context(tc.tile_pool(name=f"ps{k}", bufs=2, space="PSUM"))
            for k in range(NPAIR)
        ]

        e_sb = singles.tile([128, 64], BF16)
        nc.sync.dma_start(out=e_sb, in_=wts.ap())
        # events staging: sum rows land on partition 48 (copy window 32..48)
        stage = singles.tile([64, 4 * NEV * COLS], FP32)

        # Two persistent weight tiles (64x64 array mode): E+ones-col, both
        # stream slots.  Every matmul reuses them via tile_position.
        nc.tensor.ldweights(e_sb[0:64, :], tile_position=(0, 0))
        nc.tensor.ldweights(e_sb[64:128, :], tile_position=(64, 64))

        wt = [[None] * NCH for _ in range(NPAIR)]

        p_cur = [None] * NPAIR
        for j in range(0, H + 1):
            ch, pos = divmod(j, TCH)
            for k in range(NPAIR):
                if pos == 0 and wt[k][ch] is None:
                    w = wpools[k].tile([128, TCH * COLS], BF16, tag=f"w{k}")
                    r0 = (k * NCH + ch) * 128
                    if ch == 0:
                        # early hops unblock after the first 1/4 arrives
                        sub = TCH // 4 * COLS
                        for u in range(4):
                            nc.sync.dma_start(
                                out=w[:, u * sub : (u + 1) * sub],
                                in_=wbuf.ap()[r0 : r0 + 128, u * sub : (u + 1) * sub],
                            )
                    else:
                        nc.sync.dma_start(out=w, in_=wbuf.ap()[r0 : r0 + 128, :])
                    wt[k][ch] = w

                wsl = wt[k][ch][0:112, pos * COLS : (pos + 1) * COLS]
                if j == 0:
                    p = ppools[k].tile([128, COLS], BF16, tag=f"p{k}")
                    nc.vector.tensor_copy(p[0:112, :], wsl)
                    p_cur[k] = p
                    continue

                q = pspools[k].tile([128, COLS], FP32, tag=f"q{k}")
                nc.tensor.matmul(
                    q[0:64, :],
                    e_sb[0:48, 0:64],
                    p_cur[k][0:48, :],
                    start=True,
                    stop=True,
                ).ins.ldweights = False
                nc.tensor.matmul(
                    q[64:128, :],
                    e_sb[64:112, 0:64],
                    p_cur[k][64:112, :],
                    start=True,
                    stop=True,
                ).ins.ldweights = False

                pn = ppools[k].tile([128, COLS], BF16, tag=f"p{k}")
                nc.vector.scalar_tensor_tensor(
                    out=pn[0:112, :],
                    in0=q[0:112, :],
                    scalar=2.0 ** (-S2),
                    in1=wsl,
                    op0=mybir.AluOpType.mult,
                    op1=mybir.AluOpType.mult,
                )
                p_cur[k] = pn

                if j in EVENTS:
                    ev = EVENTS.index(j)
                    for half, base in ((0, 32), (1, 96)):
                        # stream 2k+half's colsum is q row 48/112; copy the
                        # 32-aligned window [base, base+17) -> rows 32..48.
                        sidx = 2 * k + half
                        off = (sidx * NEV + ev) * COLS
                        nc.scalar.copy(
                            stage[32:49, off : off + COLS],
                            q[base : base + 17, :],
                        )

        nc.sync.dma_start(out=out_cs.ap(), in_=stage[48:49, :])

    # Excess matmul waits must become sync-queue event semaphores, not get
    # pinned onto the startup ldweights (in-order PE queue would deadlock).
    nc.move_matmul_waits_to_ldweights = lambda: None
    nc.compile()
    return nc


def _host_prep(feats, trans, start, end):
    """Per-core input dicts: emission slices per (core, stream, hop)."""
    import ml_dtypes

    bf16 = ml_dtypes.bfloat16
    E = np.exp(trans.astype(np.float64)).astype(np.float32)
    wts = np.zeros((128, 64), np.float32)
    wts[0:48, 0:48] = E
    wts[0:48, 48] = 1.0
    wts[64:112, 0:48] = E
    wts[64:112, 48] = 1.0
    wts = wts.astype(bf16)

    in_maps = []
    for c in range(NCORES):
        sh, tau = c // 4, c % 4
        colsl = slice(sh * COLS, (sh + 1) * COLS)
        f = feats[colsl]  # [COLS, L, T] float32
        # arr[slice j, stream, tag, col]
        arr = np.ones((NCH * TCH, 4, T, COLS), np.float32)
        for sidx in range(4):
            seg = 4 * tau + sidx
            a = _seg_start(seg)
            t0 = 0 if seg == 0 else a - ETA - 1
            for j in range(NSLICE):
                t = t0 + j
                if t > L - 1:
                    continue  # padded (all ones)
                sl = f[:, t, :].astype(np.float64)
                if seg == 0 and j == 0:
                    sl = sl + start.astype(np.float64)
                if seg == NSEG - 1 and t == L - 1:
                    sl = sl + end.astype(np.float64)
                arr[j, sidx] = np.exp(sl).T.astype(np.float32)
        # device rows per (pair, chunk): stream 2k at 0-47, 2k+1 at 64-111,
        # zero padding at 48-63/112-127 (keeps sim-visible SBUF initialized
        # and NaN-free garbage lanes) -> [NPAIR, NCH, 128, TCH, COLS]
        a4 = arr.reshape(NCH, TCH, 2, 2, T, COLS).transpose(2, 0, 3, 4, 1, 5)
        full = np.zeros((NPAIR, NCH, 2, 64, TCH, COLS), np.float32)
        full[:, :, :, 0:48] = a4
        wb = (
            np.ascontiguousarray(full)
            .astype(bf16)
            .reshape(NPAIR * NCH * 128, TCH * COLS)
        )
        in_maps.append({"wbuf": wb, "wts": wts})
    return in_maps


def _host_finish(results, feats, tags, trans, start, end):
    """Assemble log Z from colsums + exact gold score; returns NLL [B]."""
    c2 = S2 * math.log(2.0)
    logz = np.zeros(B, dtype=np.float64)
    for c in range(NCORES):
        sh, tau = c // 4, c % 4
        colsl = slice(sh * COLS, (sh + 1) * COLS)
        cs = results[c]["out_cs"].reshape(4, NEV, COLS).astype(np.float64)
        for sidx in range(4):
            seg = 4 * tau + sidx
            if seg == NSEG - 1:
                lend = 63 * c2 + np.log(cs[sidx, 1])
            else:
                lend = 76 * c2 + np.log(cs[sidx, 2])
            bound = 0.0 if seg == 0 else ETA * c2 + np.log(cs[sidx, 0])
            logz[colsl] += lend - bound

    f = feats.astype(np.float64)
    emit = np.take_along_axis(f, tags[:, :, None].astype(np.int64), axis=2)[:, :, 0]
    gold = (
        emit.sum(axis=1)
        + trans.astype(np.float64)[tags[:, :-1], tags[:, 1:]].sum(axis=1)
        + start.astype(np.float64)[tags[:, 0]]
        + end.astype(np.float64)[tags[:, -1]]
    )
    return (logz - gold).astype(np.float32)


def kernel(feats, tags, mask, trans_m, start_scores, end_scores):
    feats = np.asarray(feats, dtype=np.float32)
    tags = np.asarray(tags, dtype=np.int32)
    trans_m = np.asarray(trans_m, dtype=np.float32)
    start_scores = np.asarray(start_scores, dtype=np.float32)
    end_scores = np.asarray(end_scores, dtype=np.float32)

    nc = _build()
    in_maps = _host_prep(feats, trans_m, start_scores, end_scores)
    res = run_bass_kernel_spmd(nc, in_maps, list(range(NCORES)))
    return _host_finish(res.results, feats, tags, trans_m, start_scores, end_scores)
